# revision 4
# baseline (speedup 1.0000x reference)
"""Trainium2 Bass kernel for nn_DecoderWithAttention.

One decoder step: Bahdanau-dot attention + single-step vanilla RNN + vocab
projection, distributed over 8 NeuronCores:
  - attention + RNN: data-parallel over batch (32 rows/core)
  - h_new exchanged with an on-device AllGather collective
  - fc projection: tensor-parallel over vocab (4000 cols/core), bf16 weights

kernel(**inputs) takes the FULL unsharded inputs and returns
(logits [256,32000], h_new [1,256,1024], attn [256,512]) as float32.

Hardware notes baked into the structure:
  - TensorE contracts over partitions only, so encoder rows are transposed
    on-chip (PE transpose in float32r mode, 1.5 cyc/row) for the energy
    matmul, while ctx uses the natural [s,h] layout.
  - float32r matmuls run at 1 cyc/row for N>=256 and are bit-identical to
    fp32 here; inputs must be produced by DMA or a DVE/ACT copy into a
    float32r-typed tile.
  - Engine (DVE/ACT) partition bases must be 32-aligned; single-row moves
    are done via PE transposes + free-dim column slicing, or DMA (which has
    no partition alignment rules).
"""
import numpy as np
import ml_dtypes

import concourse.bacc as bacc
import concourse.mybir as mybir
import concourse.tile as tile
from concourse.bass import IndirectOffsetOnAxis
from concourse.masks import make_identity

NC = 8          # cores
BS, S, H, E, V = 256, 512, 1024, 300, 32000
B = BS // NC    # 32 local batch rows
VL = V // NC    # 4000 local vocab cols
EH = E + H      # 1324
G = 8           # group size (rows per attnT stationary block)
F32 = mybir.dt.float32
F32R = mybir.dt.float32r
BF16 = mybir.dt.bfloat16
I32 = mybir.dt.int32
AX = mybir.AxisListType.X
AF = mybir.ActivationFunctionType
ADD = mybir.AluOpType.add

KH = H // 128   # 8 h-chunks
KS = S // 128   # 4 s-chunks
NG = B // G     # 4 groups

def _build():
    nc = bacc.Bacc(None, num_devices=NC)

    enc = nc.dram_tensor("enc", [B, S, H], F32, kind="ExternalInput")
    idx = nc.dram_tensor("idx", [B, 1], I32, kind="ExternalInput")
    hT = nc.dram_tensor("hT", [H, B], F32, kind="ExternalInput")
    emb = nc.dram_tensor("emb", [V, E], F32, kind="ExternalInput")
    wih = nc.dram_tensor("wih", [EH, H], F32, kind="ExternalInput")
    whh = nc.dram_tensor("whh", [H, H], F32, kind="ExternalInput")
    brep = nc.dram_tensor("brep", [B, H], F32, kind="ExternalInput")
    fcw = nc.dram_tensor("fcw", [H, VL], BF16, kind="ExternalInput")
    fcb = nc.dram_tensor("fcb", [128, VL], F32, kind="ExternalInput")

    logits_o = nc.dram_tensor("logits_o", [BS, VL], F32, kind="ExternalOutput")
    hnew_o = nc.dram_tensor("hnew_o", [B, H], F32, kind="ExternalOutput")
    attn_o = nc.dram_tensor("attn_o", [B, S], F32, kind="ExternalOutput")

    with tile.TileContext(nc) as tc:
        with (
            tc.tile_pool(name="const", bufs=1) as cp,
            tc.tile_pool(name="dram", bufs=1, space="DRAM") as dp,
        ):
            # ---------------- constants / setup ----------------
            ident = cp.tile([128, 128], F32, name="ident")
            make_identity(nc, ident[:])
            identr = cp.tile([128, 128], F32R, name="identr")
            nc.vector.tensor_copy(identr[:], ident[:])

            ht = []
            for k in range(KH):
                t = cp.tile([128, B], F32R, name=f"ht{k}", tag=f"ht{k}")
                nc.sync.dma_start(t[:], hT[k * 128:(k + 1) * 128, :].bitcast(F32R))
                ht.append(t)

            brep_sb = cp.tile([B, H], F32, name="brep_sb")
            nc.sync.dma_start(brep_sb[:], brep[:])
            idx_sb = cp.tile([B, 1], I32, name="idx_sb")
            nc.sync.dma_start(idx_sb[:], idx[:])
            gat = cp.tile([B, E], F32, name="gat")
            nc.gpsimd.indirect_dma_start(
                out=gat[:], out_offset=None, in_=emb[:],
                in_offset=IndirectOffsetOnAxis(ap=idx_sb[:, :1], axis=0),
            )
            fcb_sb = cp.tile([128, VL], F32, name="fcb_sb")
            nc.sync.dma_start(fcb_sb[:], fcb[:])

            # ctxT_all [128, KH*B]: h-chunk k lives at cols [k*B, (k+1)*B)
            ctxT_all = cp.tile([128, KH * B], F32R, name="ctxT_all")

            # ---------------- attention ----------------
            with (
                tc.tile_pool(name="encp", bufs=12) as ep,
                tc.tile_pool(name="etp", bufs=10) as etp,
                tc.tile_pool(name="smallA", bufs=3) as sm,
                tc.tile_pool(name="p_tr", bufs=2, space="PSUM") as p_tr,
                tc.tile_pool(name="p_e", bufs=2, space="PSUM") as p_e,
                tc.tile_pool(name="p_ctx", bufs=2, space="PSUM") as p_ctx,
                tc.tile_pool(name="p_at", bufs=2, space="PSUM") as p_at,
            ):
                for r in range(B):
                    # enc row natural [s, h]: 4 tiles of [128, 1024]
                    en = []
                    for sc in range(KS):
                        t = ep.tile([128, H], F32R, tag="enc_nat",
                                    name=f"en{r}_{sc}")
                        nc.sync.dma_start(
                            t[:],
                            enc[r, sc * 128:(sc + 1) * 128, :].bitcast(F32R))
                        en.append(t)
                    # transpose to encT [h, s]: 8 tiles of [128, 512]
                    enT = []
                    for hc in range(KH):
                        pt = p_tr.tile([128, S], F32, tag="ptr",
                                       name=f"pt{r}_{hc}")
                        for sc in range(KS):
                            nc.tensor.transpose(
                                pt[:, sc * 128:(sc + 1) * 128].bitcast(F32R),
                                en[sc][:, hc * 128:(hc + 1) * 128],
                                identr[:])
                        et = etp.tile([128, S], F32R, tag="encT",
                                      name=f"et{r}_{hc}")
                        if hc % 2 == 0:
                            nc.vector.tensor_copy(et[:], pt[:])
                        else:
                            nc.scalar.activation(et[:], pt[:], AF.Copy)
                        enT.append(et)
                    # energy: psum [1, 512] (M=1 stationary = h column r)
                    pe = p_e.tile([1, S], F32, tag="pe", name=f"pe{r}")
                    for k in range(KH):
                        nc.tensor.matmul(pe[:], ht[k][:, r:r + 1], enT[k][:],
                                         start=(k == 0), stop=(k == KH - 1))

                    # softmax on [1, S] at partition 0
                    mx = sm.tile([1, 1], F32, name=f"mx{r}", tag="mx")
                    nmx = sm.tile([1, 1], F32, name=f"nmx{r}", tag="nmx")
                    zt = sm.tile([1, 1], F32, name=f"zt{r}", tag="zt")
                    rz = sm.tile([1, 1], F32, name=f"rz{r}", tag="rz")
                    att = sm.tile([1, S], F32, name=f"att{r}", tag="att")
                    asc = sm.tile([1, S], F32, name=f"asc{r}", tag="asc")
                    nc.vector.reduce_max(out=mx[:], in_=pe[:], axis=AX)
                    nc.vector.tensor_scalar_mul(nmx[:], mx[:], -1.0)
                    nc.scalar.activation(att[:], pe[:], AF.Exp,
                                         bias=nmx[:], scale=1.0,
                                         accum_out=zt[:])
                    nc.vector.reciprocal(rz[:], zt[:])
                    nc.vector.tensor_scalar_mul(asc[:], att[:], rz[:])
                    # attn output row (DMA: partition-base free)
                    nc.sync.dma_start(attn_o[r:r + 1, :], asc[:])

                    # attnT [s, 1] columns via PE transpose of [1, 128] chunks
                    pat = p_at.tile([128, KS], F32, tag="pext", name=f"pat{r}")
                    for sc in range(KS):
                        nc.tensor.transpose(
                            pat[:, sc:sc + 1],
                            asc[:, sc * 128:(sc + 1) * 128],
                            ident[:1, :1])
                    atT = sm.tile([128, KS], F32R, name=f"atT{r}", tag="atT")
                    nc.vector.tensor_copy(atT[:], pat[:])

                    # ctx: psum [1, 512] per h-half, accumulate over s-chunks
                    cg = sm.tile([1, H], F32, name=f"cg{r}", tag="cg")
                    for hh in range(2):
                        pc = p_ctx.tile([1, 512], F32, tag="pctx",
                                        name=f"pc{r}_{hh}")
                        for sc in range(KS):
                            nc.tensor.matmul(
                                pc[:],
                                atT[:, sc:sc + 1],
                                en[sc][:, hh * 512:(hh + 1) * 512],
                                start=(sc == 0), stop=(sc == KS - 1))
                        if hh == 0:
                            nc.vector.tensor_copy(cg[:, 0:512], pc[:])
                        else:
                            nc.scalar.activation(cg[:, 512:1024], pc[:],
                                                 AF.Copy)
                    # fold ctx row r into ctxT_all columns hc*B + r
                    pxt = p_at.tile([128, KH], F32, tag="pext", name=f"pxt{r}")
                    for hc in range(KH):
                        nc.tensor.transpose(
                            pxt[:, hc:hc + 1],
                            cg[:, hc * 128:(hc + 1) * 128],
                            ident[:1, :1])
                    nc.vector.tensor_copy(ctxT_all[:, r::B], pxt[:])

            # ---------------- RNN + AllGather + FC ----------------
            with (
                tc.tile_pool(name="sbB", bufs=1) as sb2,
                tc.tile_pool(name="wstream", bufs=4) as wp,
                tc.tile_pool(name="fcwp", bufs=4) as fp,
                tc.tile_pool(name="lgp", bufs=3) as lp,
                tc.tile_pool(name="p_x", bufs=2, space="PSUM") as p_x,
                tc.tile_pool(name="p_rnn", bufs=2, space="PSUM") as p_rnn,
                tc.tile_pool(name="p_hT", bufs=2, space="PSUM") as p_hT,
                tc.tile_pool(name="p_fc", bufs=2, space="PSUM") as p_fc,
            ):
                # embT tiles from gathered embeddings
                embT = []
                esz = [128, 128, 44]
                for c, ksz in enumerate(esz):
                    px = p_x.tile([128, B], F32, tag="px", name=f"pxe{c}")
                    nc.tensor.transpose(px[:ksz, :],
                                        gat[:, c * 128:c * 128 + ksz],
                                        ident[:B, :B])
                    t = sb2.tile([128, B], F32R, name=f"embT{c}", tag=f"embT{c}")
                    nc.vector.tensor_copy(t[:ksz, :], px[:ksz, :])
                    embT.append(t)

                # RNN: h_new = tanh(xT.T@W_ihT + hT.T@W_hhT + b)
                hnew_sb = sb2.tile([B, H], F32, name="hnew_sb")
                for half in range(2):
                    pr = p_rnn.tile([B, 512], F32, tag="prnn", name=f"pr{half}")
                    nmm = 0
                    tot = len(esz) + KH + KH
                    for c, ksz in enumerate(esz):      # emb part of W_ih
                        wt = wp.tile([128, 512], F32R, tag="wst",
                                     name=f"wi{half}_{c}")
                        nc.sync.dma_start(
                            wt[:ksz, :],
                            wih[c * 128:c * 128 + ksz,
                                half * 512:(half + 1) * 512].bitcast(F32R))
                        nc.tensor.matmul(pr[:], embT[c][:ksz, :], wt[:ksz, :],
                                         start=(nmm == 0), stop=(nmm == tot - 1))
                        nmm += 1
                    for k in range(KH):                # ctx part of W_ih
                        wt = wp.tile([128, 512], F32R, tag="wst",
                                     name=f"wc{half}_{k}")
                        nc.sync.dma_start(
                            wt[:],
                            wih[E + k * 128:E + (k + 1) * 128,
                                half * 512:(half + 1) * 512].bitcast(F32R))
                        nc.tensor.matmul(pr[:],
                                         ctxT_all[:, k * B:(k + 1) * B], wt[:],
                                         start=(nmm == 0), stop=(nmm == tot - 1))
                        nmm += 1
                    for k in range(KH):                # W_hh part
                        wt = wp.tile([128, 512], F32R, tag="wst",
                                     name=f"wh{half}_{k}")
                        nc.sync.dma_start(
                            wt[:],
                            whh[k * 128:(k + 1) * 128,
                                half * 512:(half + 1) * 512].bitcast(F32R))
                        nc.tensor.matmul(pr[:], ht[k][:], wt[:],
                                         start=(nmm == 0), stop=(nmm == tot - 1))
                        nmm += 1
                    tsum = sb2.tile([B, 512], F32, name=f"tsum{half}", tag="tsum")
                    nc.vector.tensor_tensor(
                        out=tsum[:], in0=pr[:],
                        in1=brep_sb[:, half * 512:(half + 1) * 512], op=ADD)
                    nc.scalar.activation(hnew_sb[:, half * 512:(half + 1) * 512],
                                         tsum[:], AF.Tanh)
                nc.sync.dma_start(hnew_o[:], hnew_sb[:])

                # AllGather h_new across the 8 cores (Local DRAM bounce)
                ag_in = dp.tile([B, H], F32, name="ag_in")
                ag_out = dp.tile([BS, H], F32, name="ag_out")
                nc.sync.dma_start(ag_in[:], hnew_sb[:])
                nc.gpsimd.collective_compute(
                    "AllGather", mybir.AluOpType.bypass,
                    replica_groups=[list(range(NC))],
                    ins=[ag_in.opt()], outs=[ag_out.opt()],
                )

                # transpose h_full [256, 1024] -> hTf [1024(h), 256(b)], bf16
                hf = []
                for bc in range(2):
                    t = sb2.tile([128, H], F32R, name=f"hf{bc}", tag=f"hf{bc}")
                    nc.sync.dma_start(
                        t[:], ag_out[bc * 128:(bc + 1) * 128, :].bitcast(F32R))
                    hf.append(t)
                hTf = []
                for hc in range(KH):
                    ph = p_hT.tile([128, BS], F32, tag="phT", name=f"ph{hc}")
                    for bc in range(2):
                        nc.tensor.transpose(
                            ph[:, bc * 128:(bc + 1) * 128].bitcast(F32R),
                            hf[bc][:, hc * 128:(hc + 1) * 128],
                            identr[:])
                    t = sb2.tile([128, BS], BF16, name=f"hTf{hc}", tag=f"hTf{hc}")
                    nc.vector.tensor_copy(t[:], ph[:])
                    hTf.append(t)

                # FC: logits [256, 4000] = hTf.T @ fcw + fcb
                for n in range((VL + 511) // 512):
                    nsz = min(512, VL - n * 512)
                    fcts = []
                    for k in range(KH):
                        ft = fp.tile([128, 512], BF16, tag="fct",
                                     name=f"fc{n}_{k}")
                        nc.sync.dma_start(
                            ft[:, :nsz],
                            fcw[k * 128:(k + 1) * 128, n * 512:n * 512 + nsz])
                        fcts.append(ft)
                    for m in range(2):
                        pf = p_fc.tile([128, 512], F32, tag="pfc",
                                       name=f"pf{n}_{m}")
                        for k in range(KH):
                            nc.tensor.matmul(
                                pf[:, :nsz],
                                hTf[k][:, m * 128:(m + 1) * 128],
                                fcts[k][:, :nsz],
                                start=(k == 0), stop=(k == KH - 1))
                        lg = lp.tile([128, 512], F32, tag="lg", name=f"lg{n}_{m}")
                        nc.vector.tensor_tensor(
                            out=lg[:, :nsz], in0=pf[:, :nsz],
                            in1=fcb_sb[:, n * 512:n * 512 + nsz], op=ADD)
                        nc.sync.dma_start(
                            logits_o[m * 128:(m + 1) * 128,
                                     n * 512:n * 512 + nsz],
                            lg[:, :nsz])
    nc.finalize()
    return nc


_STATE = {}


def _exec(in_maps):
    """Execute the bass program on the 8 axon cores via PJRT shard_map.

    Mirrors concourse.bass2jax.run_bass_via_pjrt, but caches the jitted
    callable and device-resident inputs so the harness can re-time execution
    (see timed_runs).
    """
    import jax
    from jax.sharding import Mesh, PartitionSpec, NamedSharding
    from jax.experimental.shard_map import shard_map
    import concourse.mybir as _mybir
    from concourse import bass2jax

    bass2jax.install_neuronx_cc_hook()
    nc = _build()

    partition_name = (nc.partition_id_tensor.name
                      if nc.partition_id_tensor else None)
    in_names, out_names, out_avals, zero_outs = [], [], [], []
    for alloc in nc.m.functions[0].allocations:
        if not isinstance(alloc, _mybir.MemoryLocationSet):
            continue
        name = alloc.memorylocations[0].name
        if alloc.kind == "ExternalInput":
            if name != partition_name:
                in_names.append(name)
        elif alloc.kind == "ExternalOutput":
            out_names.append(name)
            shape = tuple(alloc.tensor_shape)
            dtype = _mybir.dt.np(alloc.dtype)
            out_avals.append(jax.core.ShapedArray(shape, dtype))
            zero_outs.append(np.zeros(shape, dtype))
    n_params = len(in_names)
    n_outs = len(out_avals)
    all_names = list(in_names) + out_names
    if partition_name is not None:
        all_names.append(partition_name)

    def _body(*args):
        operands = list(args)
        if partition_name is not None:
            operands.append(bass2jax.partition_id_tensor())
        outs = bass2jax._bass_exec_p.bind(
            *operands,
            out_avals=tuple(out_avals),
            in_names=tuple(all_names),
            out_names=tuple(out_names),
            lowering_input_output_aliases=(),
            sim_require_finite=True,
            sim_require_nnan=True,
            nc=nc,
        )
        return tuple(outs)

    devices = jax.devices()[:NC]
    mesh = Mesh(np.asarray(devices), ("core",))
    in_specs = (PartitionSpec("core"),) * (n_params + n_outs)
    out_specs = (PartitionSpec("core"),) * n_outs
    sharded = jax.jit(
        shard_map(_body, mesh=mesh, in_specs=in_specs, out_specs=out_specs,
                  check_rep=False),
        keep_unused=True,
    )
    sh = NamedSharding(mesh, PartitionSpec("core"))
    concat_in = [
        np.concatenate([np.asarray(in_maps[c][k]) for c in range(NC)], axis=0)
        for k in in_names
    ]
    concat_zeros = [np.zeros((NC * z.shape[0], *z.shape[1:]), z.dtype)
                    for z in zero_outs]
    dev_in = [jax.device_put(a, sh) for a in concat_in]
    dev_zeros = [jax.device_put(a, sh) for a in concat_zeros]
    out_arrs = sharded(*dev_in, *dev_zeros)
    jax.block_until_ready(out_arrs)

    _STATE.update(sharded=sharded, dev_in=dev_in, dev_zeros=dev_zeros,
                  out_names=out_names, out_avals=out_avals)
    return [
        {name: np.asarray(out_arrs[i]).reshape(NC, *out_avals[i].shape)[c]
         for i, name in enumerate(out_names)}
        for c in range(NC)
    ]


def timed_runs(n=5):
    """Re-execute the last-built kernel n times; returns wall ns per run."""
    import time
    import jax
    sharded = _STATE["sharded"]
    dev_in = _STATE["dev_in"]
    dev_zeros = _STATE["dev_zeros"]
    times = []
    for _ in range(n):
        t0 = time.perf_counter_ns()
        out = sharded(*dev_in, *dev_zeros)
        jax.block_until_ready(out)
        times.append(time.perf_counter_ns() - t0)
    return times


def kernel(current_indices, hiddens_from_prev_step, encoder_outputs,
           emb_table, W_ih, b_ih, W_hh, b_hh, fc_W, fc_b):
    idx = np.asarray(current_indices).astype(np.int32).reshape(BS, 1)
    h2 = np.asarray(hiddens_from_prev_step, dtype=np.float32)[0]      # [256,1024]
    hT = np.ascontiguousarray(h2.T)                                    # [1024,256]
    enc = np.ascontiguousarray(np.asarray(encoder_outputs, dtype=np.float32))
    embt = np.ascontiguousarray(np.asarray(emb_table, dtype=np.float32))
    W_ihT = np.ascontiguousarray(np.asarray(W_ih, dtype=np.float32).T)  # [1324,1024]
    W_hhT = np.ascontiguousarray(np.asarray(W_hh, dtype=np.float32).T)  # [1024,1024]
    bsum = (np.asarray(b_ih, dtype=np.float32)
            + np.asarray(b_hh, dtype=np.float32))                      # [1024]
    brep = np.ascontiguousarray(np.tile(bsum[None, :], (B, 1)))        # [32,1024]
    fcwT = np.asarray(fc_W, dtype=np.float32).T.astype(ml_dtypes.bfloat16)
    fcwT = np.ascontiguousarray(fcwT)                                  # [1024,32000]
    fcb = np.asarray(fc_b, dtype=np.float32)                           # [32000]

    in_maps = []
    for c in range(NC):
        bs, be = c * B, (c + 1) * B
        vs, ve = c * VL, (c + 1) * VL
        in_maps.append({
            "enc": np.ascontiguousarray(enc[bs:be]),
            "idx": np.ascontiguousarray(idx[bs:be]),
            "hT": np.ascontiguousarray(hT[:, bs:be]),
            "emb": embt,
            "wih": W_ihT,
            "whh": W_hhT,
            "brep": brep,
            "fcw": np.ascontiguousarray(fcwT[:, vs:ve]),
            "fcb": np.ascontiguousarray(
                np.tile(fcb[None, vs:ve], (128, 1)).astype(np.float32)),
        })

    results = _exec(in_maps)

    logits = np.concatenate([results[c]["logits_o"] for c in range(NC)], axis=1)
    h_new = np.concatenate([results[c]["hnew_o"] for c in range(NC)],
                           axis=0)[None]
    attn = np.concatenate([results[c]["attn_o"] for c in range(NC)], axis=0)
    return logits, h_new, attn
